# revision 17
# baseline (speedup 1.0000x reference)
"""Multi-head attention TRN2 Bass kernel (B=4, S=2048, E=2048, H=16, D=128).

Sharding: 2 heads per core (tensor parallel over H=16 across 8 cores).
Each core computes q/k/v projections for its 2 heads over all batches,
attention, and a partial out-projection (its heads' columns of W_out).
Host sums the 8 fp32 partial outputs (the "all-reduce") and transposes.

Schedule: software-pipelined emission. The attention inner loop is
ACT-engine-paced (exp), so QKV matmuls for batch b+1 and out-projection
matmuls for batch b are interleaved into the PE queue as filler to keep
the tensor engine dense. PSUM budget (8 banks): scores 2x[128,1024] (4),
attnV accumulators 2x[128,512] (2), shared filler pool 2x[128,512] (2).

Device layouts (per core):
  xt   [B, E, S]  bf16   x transposed per batch (feature-major)
  wqk  [E, 4D]    bf16   W_q/W_k columns for heads (q0|q1|k0|k1)
  wv   [E, 2D]    bf16   W_v columns (v0|v1)
  wo   [2D, E]    bf16   W_out^T rows for this core's head channels
  yt   [B, E, S]  f16    partial output, feature-major (ExternalOutput)
"""

import os
import sys

sys.path.insert(0, "/opt/trn_rl_repo")

import numpy as np
import ml_dtypes

B, S, E = 4, 2048, 2048
H, D = 16, 128
NCORES = 8
HPC = H // NCORES  # 2 heads per core
SCALE = 1.0 / float(np.sqrt(D))

EC = E // 128     # 16 contraction chunks
TCX = 1024        # x sbuf tile width (tokens)
NTCX = S // TCX   # 2
KC = S // 128     # 16 key chunks
NQP = S // 1024   # 2 query chunks


class Filler:
    """Round-robin multiplexer over (generator, remaining-count) streams.
    pull() rotates across streams so short out-proj chains interleave with
    long qkv chains (hiding PSUM drain turnaround); drain_keep() empties
    in FIFO order so earlier streams (qkv for the next batch) are always
    fully emitted before the attention that reads their tiles."""

    def __init__(self):
        self.streams = []
        self.idx = 0

    def add(self, gen, n_units):
        self.streams.append([gen, n_units])

    def remaining(self):
        return sum(s[1] for s in self.streams)

    def _pull_from(self, i):
        g = self.streams[i]
        try:
            next(g[0])
            g[1] -= 1
            return True
        except StopIteration:
            self.streams.pop(i)
            if self.idx > i:
                self.idx -= 1
            return False

    def pull(self, n=1):
        while n > 0 and self.streams:
            if self.idx >= len(self.streams):
                self.idx = 0
            if self._pull_from(self.idx):
                self.idx += 1
                n -= 1

    def drain_keep(self, keep=0):
        while self.remaining() > keep and self.streams:
            if self._pull_from(0):
                pass


def _build():
    import concourse.bass as bass
    import concourse.tile as tile
    from concourse import bacc, mybir

    bf = mybir.dt.bfloat16
    f16 = mybir.dt.float16
    f32 = mybir.dt.float32
    ADD = mybir.AluOpType.add
    MULT = mybir.AluOpType.mult
    EXP = mybir.ActivationFunctionType.Exp

    nc = bacc.Bacc(
        "TRN2", target_bir_lowering=False, debug=False, num_devices=NCORES
    )
    xt = nc.dram_tensor("xt", [B, E, S], bf, kind="ExternalInput").ap()
    wqk = nc.dram_tensor("wqk", [E, 4 * D], bf, kind="ExternalInput").ap()
    wv = nc.dram_tensor("wv", [E, 2 * D], bf, kind="ExternalInput").ap()
    wo = nc.dram_tensor("wo", [2 * D, E], bf, kind="ExternalInput").ap()
    yt = nc.dram_tensor("yt", [B, E, S], f16, kind="ExternalOutput").ap()

    with tile.TileContext(nc) as tc:
        with (
            tc.tile_pool(name="wp", bufs=1) as wp,
            tc.tile_pool(name="xp", bufs=24) as xp,
            tc.tile_pool(name="qkp", bufs=8) as qkp,
            tc.tile_pool(name="vp", bufs=4) as vp,
            tc.tile_pool(name="ptp", bufs=8) as ptp,
            tc.tile_pool(name="accp", bufs=2) as accp,
            tc.tile_pool(name="recp", bufs=2) as recp,
            tc.tile_pool(name="dbp", bufs=2) as dbp,
            tc.tile_pool(name="osp", bufs=4) as osp,
            tc.tile_pool(name="yp", bufs=4) as yp,
            tc.tile_pool(name="scp", bufs=2, space="PSUM") as scp,
            tc.tile_pool(name="ocp", bufs=2, space="PSUM") as ocp,
            tc.tile_pool(name="fpp", bufs=2, space="PSUM") as fpp,
        ):
            wqk_sb = wp.tile([128, EC * 4 * D], bf, tag="wqk", name="wqk_sb")
            wv_sb = wp.tile([128, EC * 2 * D], bf, tag="wv", name="wv_sb")
            wo_sb = wp.tile([128, 2 * E], bf, tag="wo", name="wo_sb")
            ones_sb = wp.tile([128, 1], f16, tag="ones", name="ones_sb")
            nc.vector.memset(ones_sb, 1.0)

            # qk_tiles[b % 2][mc]: mc in (q_h0, q_h1, k_h0, k_h1), [D, S]
            qk_tiles = {}
            # v_sb[(b % 2, tcx)]: [128 tokens, 8 tsub * 256 chans] f16
            v_tiles = {}
            out_sb = {}  # (b % 2, h) -> [128, S] bf16

            def emit_x_dma(b, tcx):
                xs = []
                for ec in range(EC):
                    t = xp.tile([128, TCX], bf, tag="x", name=f"x_{b}_{tcx}_{ec}")
                    nc.sync.dma_start(
                        t,
                        xt[b, ec * 128 : (ec + 1) * 128,
                           tcx * TCX : (tcx + 1) * TCX],
                    )
                    xs.append(t)
                return xs

            def qkv_stream(b, x0=None):
                """Units: [x0 dma], qk+v chains tcx0, x1 dma, qk+v chains
                tcx1. Unit count: (1 if x0 is None else 0) + 76 + 1 + 76."""
                for mc in range(4):
                    qk_tiles[(b % 2, mc)] = qkp.tile(
                        [128, S], bf, tag="qk", name=f"qk_{b}_{mc}"
                    )
                for tcx in range(NTCX):
                    v_tiles[(b % 2, tcx)] = vp.tile(
                        [128, 8 * 2 * D], f16, tag="vsb", name=f"v_{b}_{tcx}"
                    )
                xs = [None, None]
                if x0 is not None:
                    xs[0] = x0
                else:
                    xs[0] = emit_x_dma(b, 0)
                    yield
                # per tcx: q/k projection chains then v chains (v chains
                # release the x tiles before the next tcx's DMA needs slots)
                for tcx in range(NTCX):
                    if tcx == 1:
                        xs[1] = emit_x_dma(b, 1)
                        yield
                    for mc in range(4):
                        for tcs in range(2):
                            pq = fpp.tile(
                                [128, 512], f32, tag="fp",
                                name=f"pq_{b}_{tcx}_{mc}_{tcs}",
                            )
                            for ecg in range(4):
                                for ei in range(4):
                                    ec = ecg * 4 + ei
                                    nc.tensor.matmul(
                                        pq,
                                        lhsT=wqk_sb[
                                            :, ec * 512 + mc * 128
                                            : ec * 512 + (mc + 1) * 128
                                        ],
                                        rhs=xs[tcx][ec][
                                            :, tcs * 512 : (tcs + 1) * 512
                                        ],
                                        start=(ec == 0),
                                        stop=(ec == EC - 1),
                                    )
                                yield
                            tchunk = tcx * TCX + tcs * 512
                            nc.vector.tensor_copy(
                                qk_tiles[(b % 2, mc)][:, tchunk : tchunk + 512],
                                pq,
                            )
                            yield
                    # v projection chains (pairs of tsub share one PSUM slot)
                    for tp in range(4):
                        pv = fpp.tile(
                            [128, 512], f32, tag="fp", name=f"pv_{b}_{tcx}_{tp}"
                        )
                        for half in range(2):
                            tsub = tp * 2 + half
                            for ecg in range(4):
                                for ei in range(4):
                                    ec = ecg * 4 + ei
                                    nc.tensor.matmul(
                                        pv[:, half * 256 : (half + 1) * 256],
                                        lhsT=xs[tcx][ec][
                                            :, tsub * 128 : (tsub + 1) * 128
                                        ],
                                        rhs=wv_sb[:, ec * 256 : (ec + 1) * 256],
                                        start=(ec == 0),
                                        stop=(ec == EC - 1),
                                    )
                                yield
                        nc.vector.tensor_copy(
                            v_tiles[(b % 2, tcx)][:, tp * 512 : (tp + 1) * 512],
                            pv,
                        )
                        yield

            def outproj_stream(b, qp):
                """Unit count: 16 fc * (2 + 1) = 48."""
                for fc in range(E // 128):
                    y_sb = yp.tile(
                        [128, 1024], f16, tag="y", name=f"y_{b}_{qp}_{fc}"
                    )
                    for th in range(2):
                        yps = fpp.tile(
                            [128, 512], f32, tag="fp", name=f"yps_{b}_{qp}_{fc}_{th}"
                        )
                        tok = qp * 1024 + th * 512
                        for cc in range(HPC):
                            nc.tensor.matmul(
                                yps,
                                lhsT=wo_sb[
                                    :, cc * E + fc * 128 : cc * E + (fc + 1) * 128
                                ],
                                rhs=out_sb[(b % 2, cc)][:, tok : tok + 512],
                                start=(cc == 0),
                                stop=(cc == HPC - 1),
                            )
                        yield
                        nc.any.tensor_copy(y_sb[:, th * 512 : (th + 1) * 512], yps)
                    nc.sync.dma_start(
                        yt[b, fc * 128 : (fc + 1) * 128,
                           qp * 1024 : (qp + 1) * 1024],
                        y_sb,
                    )
                    yield

            def emit_attention(b, filler):
                for h in range(HPC):
                    out_sb[(b % 2, h)] = osp.tile(
                        [128, S], bf, tag="osb", name=f"osb_{b}_{h}"
                    )
                for qp in range(NQP):
                    for h in range(HPC):
                        q_t = qk_tiles[(b % 2, h)]
                        k_t = qk_tiles[(b % 2, 2 + h)]
                        q0 = qp * 1024
                        out_a = ocp.tile(
                            [128, 512], f32, tag="oc", name=f"oa_{b}_{h}_{qp}"
                        )
                        out_b = ocp.tile(
                            [128, 512], f32, tag="oc", name=f"ob_{b}_{h}_{qp}"
                        )
                        acc = accp.tile(
                            [128, 1024], f16, tag="acc", name=f"acc_{b}_{h}_{qp}"
                        )
                        def emit_scores(kc):
                            sps = scp.tile(
                                [128, 1024], f32, tag="sc",
                                name=f"s_{b}_{h}_{qp}_{kc}",
                            )
                            nc.tensor.matmul(
                                sps[:, :512],
                                lhsT=k_t[:, kc * 128 : (kc + 1) * 128],
                                rhs=q_t[:, q0 : q0 + 512],
                                start=True,
                                stop=True,
                            )
                            nc.tensor.matmul(
                                sps[:, 512:],
                                lhsT=k_t[:, kc * 128 : (kc + 1) * 128],
                                rhs=q_t[:, q0 + 512 : q0 + 1024],
                                start=True,
                                stop=True,
                            )
                            return sps

                        pt_prev = None
                        sps_next = emit_scores(0)
                        for kc in range(KC):
                            sps = sps_next
                            pt = ptp.tile(
                                [128, 1024], f16, tag="pt",
                                name=f"pt_{b}_{h}_{qp}_{kc}",
                            )
                            nc.scalar.activation(pt, sps, EXP, scale=SCALE)
                            # scores(kc+1) + a filler unit go between exp(kc)
                            # and attnV(kc) in the PE queue, covering the exp
                            # latency so attnV never stalls the PE.
                            if kc + 1 < KC:
                                sps_next = emit_scores(kc + 1)
                            filler.pull(1)
                            vt = v_tiles[(b % 2, kc // 8)]
                            vcol = (kc % 8) * 256 + h * 128
                            nc.tensor.matmul(
                                out_a,
                                lhsT=vt[:, vcol : vcol + 128],
                                rhs=pt[:, :512],
                                start=(kc == 0),
                                stop=(kc == KC - 1),
                            )
                            nc.tensor.matmul(
                                out_b,
                                lhsT=vt[:, vcol : vcol + 128],
                                rhs=pt[:, 512:],
                                start=(kc == 0),
                                stop=(kc == KC - 1),
                            )
                            if kc == 1:
                                nc.vector.tensor_tensor(acc, pt_prev, pt, ADD)
                            elif kc > 1:
                                nc.vector.tensor_tensor(acc, acc, pt, ADD)
                            pt_prev = pt
                        # softmax denominator -> reciprocal -> broadcast
                        rec_sb = recp.tile(
                            [1, 1024], f32, tag="rec", name=f"rec_{b}_{h}_{qp}"
                        )
                        den = fpp.tile(
                            [128, 512], f32, tag="fp", name=f"den_{b}_{h}_{qp}"
                        )
                        for dh in range(2):
                            nc.tensor.matmul(
                                den[0:1, :],
                                lhsT=ones_sb,
                                rhs=acc[:, dh * 512 : (dh + 1) * 512],
                                start=True,
                                stop=True,
                            )
                            nc.vector.reciprocal_approx_fast(
                                out=rec_sb[:, dh * 512 : (dh + 1) * 512],
                                in_=den[0:1, :],
                            )
                        dbc = dbp.tile(
                            [128, 1024], f32, tag="dbc", name=f"dbc_{b}_{h}_{qp}"
                        )
                        nc.gpsimd.partition_broadcast(dbc, rec_sb)
                        # normalize directly from PSUM accumulators
                        nc.vector.tensor_tensor(
                            out_sb[(b % 2, h)][:, q0 : q0 + 512],
                            out_a, dbc[:, :512], MULT,
                        )
                        nc.vector.tensor_tensor(
                            out_sb[(b % 2, h)][:, q0 + 512 : q0 + 1024],
                            out_b, dbc[:, 512:], MULT,
                        )
                        filler.pull(2)
                    filler.add(outproj_stream(b, qp), 48)

            # ================= prologue =================
            # Interleave wqk chunk / x(b0,tcx0) tile DMAs so the first
            # projection chain starts as soon as chunk 0 lands.
            x0 = []
            for ec in range(EC):
                nc.sync.dma_start(
                    wqk_sb[:, ec * 512 : (ec + 1) * 512],
                    wqk[ec * 128 : (ec + 1) * 128, :],
                )
                t = xp.tile([128, TCX], bf, tag="x", name=f"x_0_0_{ec}")
                nc.sync.dma_start(
                    t, xt[0, ec * 128 : (ec + 1) * 128, 0:TCX]
                )
                x0.append(t)
            for ec in range(EC):
                nc.sync.dma_start(
                    wv_sb[:, ec * 256 : (ec + 1) * 256],
                    wv[ec * 128 : (ec + 1) * 128, :],
                )
            for cc in range(2):
                nc.sync.dma_start(
                    wo_sb[:, cc * E : (cc + 1) * E],
                    wo[cc * 128 : (cc + 1) * 128, :],
                )

            f = Filler()
            f.add(qkv_stream(0, x0=x0), 153)
            # batch-0 qkv must be fully emitted before attention(0) reads
            # its tiles (deps only point backward in emission order).
            f.drain_keep(0)

            for b in range(B):
                if b + 1 < B:
                    f.add(qkv_stream(b + 1), 154)
                emit_attention(b, f)
                # dense filler window at batch end; before the last batch
                # keep some units so attention(3) has PE filler.
                f.drain_keep(36 if b == B - 2 else 0)

    nc.compile()
    return nc


_NC_CACHE = None
LAST_EXEC_NS = None


def _ensure_trace_hook_stub():
    """If the image's antenv lacks axon_hooks, a stray BASS_TRACE env var
    would crash run_bass_kernel_spmd on import. Register a None-hook stub
    (concourse then logs a warning and runs without tracing)."""
    try:
        import antenv.axon_hooks  # noqa: F401
    except ImportError:
        import types

        mod = types.ModuleType("antenv.axon_hooks")
        mod.get_axon_ntff_profile_hook = lambda: None
        mod.set_axon_ntff_profile_hook = lambda h: None
        sys.modules["antenv.axon_hooks"] = mod


def kernel(**inputs):
    global _NC_CACHE, LAST_EXEC_NS
    _ensure_trace_hook_stub()
    from concourse import bass_utils

    x = np.asarray(inputs["x"], dtype=np.float32)
    w_qkv = np.asarray(inputs["w_qkv"], dtype=np.float32)
    w_out = np.asarray(inputs["w_out"], dtype=np.float32)

    bf = ml_dtypes.bfloat16
    xt_np = np.ascontiguousarray(x.transpose(0, 2, 1)).astype(bf)  # [B, E, S]

    in_maps = []
    for c in range(NCORES):
        rows_q, rows_k, rows_v = [], [], []
        for h in (HPC * c, HPC * c + 1):
            base = h * 3 * D
            rows_q.append(w_qkv[base : base + D])
            rows_k.append(w_qkv[base + D : base + 2 * D])
            rows_v.append(w_qkv[base + 2 * D : base + 3 * D])
        wqk_c = np.concatenate(rows_q + rows_k, axis=0).T  # [E, 4D]
        wv_c = np.concatenate(rows_v, axis=0).T            # [E, 2D]
        wo_c = w_out[:, HPC * c * D : (HPC * c + HPC) * D].T  # [2D, E]
        in_maps.append(
            {
                "xt": xt_np,
                "wqk": np.ascontiguousarray(wqk_c).astype(bf),
                "wv": np.ascontiguousarray(wv_c).astype(bf),
                "wo": np.ascontiguousarray(wo_c).astype(bf),
            }
        )

    if _NC_CACHE is None:
        _NC_CACHE = _build()
    nc = _NC_CACHE

    res = bass_utils.run_bass_kernel_spmd(
        nc, in_maps, core_ids=list(range(NCORES))
    )
    LAST_EXEC_NS = res.exec_time_ns

    y_t = res.results[0]["yt"].astype(np.float32)
    for c in range(1, NCORES):
        y_t += res.results[c]["yt"].astype(np.float32)
    return np.ascontiguousarray(y_t.transpose(0, 2, 1)).astype(np.float32)
